# revision 2
# baseline (speedup 1.0000x reference)
"""Hybrid split-K bf16 + fp8(e4m3 DoubleRow) grouped GEMM for TRN2.

y[t] = x[t] @ W[g].T with the contraction split: first KB k-blocks (128 each)
in bf16 (1 cy/row), last KF_PAIRS k-pairs (256 each) in fp8 e4m3 DoubleRow
(2x FLOP rate, 157 TF/s). All products accumulate in the same PSUM banks at a
common power-of-2 scale 4096*y:
    bf16 side: lhsT = bf16(4096*x)  (exact pow2, equals 4096*bf16(x)),
               rhs  = raw bf16 W^T tiles (DMA'd straight to SBUF, no cast)
    fp8 side:  lhsT = e4m3(16*x), rhs = e4m3(256*W^T)    (16*256 = 4096)
One ACT copy with scale 2^-12 evacuates PSUM (exact).

Quantization error (deterministic for the fixed jax key-0 inputs) comes only
from the fp8 fraction: rel_err = 3.15e-2 * sqrt(KF_PAIRS/16).

Sharding (8 cores): expert-parallel x output-column-parallel, same as the
bf16 baseline; host-side transforms are layout-only.
"""

import os
import sys

import numpy as np

NUM_EXPERTS = 4
GROUP = 4096
HIDDEN = 4096
TOTAL = NUM_EXPERTS * GROUP
N_CORES = 8
O_HALF = HIDDEN // 2

P = 128
IB = HIDDEN // P          # 32 k-blocks
KF_PAIRS = 4              # fp8 k-pairs (256 k each) at the tail of K
KB = IB - 2 * KF_PAIRS    # bf16 k-blocks
NB = 512                  # bf16 matmul moving free dim (one PSUM bank)
OB = O_HALF // NB         # 4 psum banks per token block
U = 256                   # tokens per pair slab
SB = 4096.0               # bf16 x scale
SX = 16.0                 # fp8 x scale
SW = 256.0                # fp8 W scale


def _ensure_paths():
    for p in ("/opt/trn_rl_repo", "/root/.axon_site", "/root/.axon_site/_ro/pypackages"):
        if os.path.isdir(p) and p not in sys.path:
            sys.path.append(p)
    try:
        import concourse  # noqa: F401
    except ImportError:
        raise RuntimeError("concourse not importable; check PYTHONPATH")


_NC_CACHE = {}


def build_nc(tb_count=GROUP // P):
    if tb_count in _NC_CACHE:
        return _NC_CACHE[tb_count]
    _ensure_paths()
    import concourse.mybir as mybir
    import concourse.tile as tile
    from concourse import bacc

    DR = mybir.MatmulPerfMode.DoubleRow
    assert tb_count % 2 == 0
    n_pairs = tb_count // 2
    CHUNKS = (2, 6, 12, 12)
    assert sum(CHUNKS) == IB

    nc = bacc.Bacc("TRN2", target_bir_lowering=False, debug=False)
    x_d = nc.dram_tensor("x", [n_pairs, HIDDEN, U], mybir.dt.float32, kind="ExternalInput")
    w_d = nc.dram_tensor("w", [HIDDEN, O_HALF], mybir.dt.bfloat16, kind="ExternalInput")
    y_d = nc.dram_tensor("y", [tb_count * P, O_HALF], mybir.dt.float32, kind="ExternalOutput")
    x_ap, w_ap, y_ap = x_d.ap(), w_d.ap(), y_d.ap()

    with tile.TileContext(nc) as tc:
        from contextlib import ExitStack

        with ExitStack() as ctx:
            wt_pool = ctx.enter_context(tc.tile_pool(name="wt", bufs=1))
            w8s_pool = ctx.enter_context(tc.tile_pool(name="w8s", bufs=2))
            w8_pool = ctx.enter_context(tc.tile_pool(name="w8", bufs=1))
            xf_pool = ctx.enter_context(tc.tile_pool(name="xf", bufs=2))
            xb_pool = ctx.enter_context(tc.tile_pool(name="xb", bufs=2))
            x8_pool = ctx.enter_context(tc.tile_pool(name="x8", bufs=2))
            out_pool = ctx.enter_context(tc.tile_pool(name="out", bufs=2))
            psum_pool = ctx.enter_context(tc.tile_pool(name="psum", bufs=1, space="PSUM"))

            # ---- W bf16 resident: KB tiles [128, O_HALF], raw DMA, no cast ----
            wT = []
            for ib in range(KB):
                t = wt_pool.tile([P, O_HALF], mybir.dt.bfloat16, name=f"wT{ib}", tag=f"wT{ib}")
                nc.scalar.dma_start(t[:], w_ap[ib * P : (ib + 1) * P, :])
                wT.append(t)
            # ---- W fp8 tail: staging DMA + DVE scaled cast ----
            w8st = []
            for j in range(KF_PAIRS):
                st = w8s_pool.tile([P, 2, O_HALF], mybir.dt.bfloat16, name=f"w8s{j}", tag="w8s")
                r0 = (KB + 2 * j) * P
                nc.scalar.dma_start(st[:], w_ap[r0 : r0 + 2 * P, :].rearrange("(j p) o -> p j o", p=P))
                w8st.append(st)
            w8 = []

            def cast_w8():
                for j in range(KF_PAIRS):
                    t = w8_pool.tile([P, 2, O_HALF], mybir.dt.float8e4, name=f"w8_{j}", tag=f"w8_{j}")
                    nc.vector.tensor_scalar_mul(t[:], w8st[j][:], SW)
                    w8.append(t)

            def load_cast(pr):
                """DMA one pair slab (ib-chunked), split-cast: bf16(4096x) head,
                e4m3(16x) tail."""
                src = x_ap[pr].rearrange("(ib p) u -> p ib u", p=P)
                xb = xb_pool.tile([P, KB, U], mybir.dt.bfloat16, name=f"xb_{pr}", tag="xb")
                x8 = x8_pool.tile([P, 2 * KF_PAIRS, U], mybir.dt.float8e4, name=f"x8_{pr}", tag="x8")
                eng = nc.scalar if pr == 1 else nc.sync
                s0 = 0
                for c, ch in enumerate(CHUNKS):
                    xf = xf_pool.tile([P, max(CHUNKS), U], mybir.dt.float32, name=f"xf_{pr}_{c}", tag="xf")
                    eng.dma_start(xf[:, :ch, :], src[:, s0 : s0 + ch, :])
                    if s0 < KB:
                        b_end = min(KB, s0 + ch)
                        nc.vector.tensor_scalar_mul(xb[:, s0:b_end, :], xf[:, : b_end - s0, :], SB)
                    if s0 + ch > KB:
                        f0 = max(KB, s0)
                        nc.vector.tensor_scalar_mul(
                            x8[:, f0 - KB : s0 + ch - KB, :], xf[:, f0 - s0 : ch, :], SX
                        )
                    s0 += ch
                return xb, x8

            def alloc_psum(tb):
                grp = (tb % 2) * OB
                return [
                    psum_pool.tile([P, NB], mybir.dt.float32, name=f"ps_{tb}_{ob}", tag=f"bank{grp + ob}")
                    for ob in range(OB)
                ]

            def mm_bf16(ps, xb, ib, t):
                lhsT = xb[:, ib, t * P : (t + 1) * P]
                for ob in range(OB):
                    nc.tensor.matmul(
                        ps[ob][:], lhsT, wT[ib][:, ob * NB : (ob + 1) * NB],
                        start=(ib == 0), stop=False,
                    )

            def mm_fp8(ps, x8, j, t):
                lhsT = x8[:, 2 * j : 2 * j + 2, t * P : (t + 1) * P]
                last = j == KF_PAIRS - 1
                for ob in range(OB):
                    for h in range(2):
                        nc.tensor.matmul(
                            ps[ob][:, h * 256 : (h + 1) * 256],
                            lhsT,
                            w8[j][:, :, (2 * ob + h) * 256 : (2 * ob + h + 1) * 256],
                            start=False,
                            stop=(last and h == 1),
                            perf_mode=DR,
                        )

            def evac_store(tb, ps):
                yo = out_pool.tile([P, O_HALF], mybir.dt.float32, name=f"yo_{tb}", tag="yo")
                for ob in range(OB):
                    nc.scalar.mul(yo[:, ob * NB : (ob + 1) * NB], ps[ob][:], 1.0 / SB)
                nc.sync.dma_start(y_ap[tb * P : (tb + 1) * P, :], yo[:])

            for pr in range(n_pairs):
                xb, x8 = load_cast(pr)
                if pr == 0:
                    cast_w8()  # DVE: after pair-0 x casts, before pair-1
                ps0 = alloc_psum(2 * pr)
                ps1 = alloc_psum(2 * pr + 1)
                if pr == 0:
                    # K-major interleave so PE weight-tile consumption matches
                    # the W stream arrival rate during the HBM-bound prologue.
                    for ib in range(KB):
                        mm_bf16(ps0, xb, ib, 0)
                        mm_bf16(ps1, xb, ib, 1)
                    for j in range(KF_PAIRS):
                        mm_fp8(ps0, x8, j, 0)
                        mm_fp8(ps1, x8, j, 1)
                else:
                    for t, ps in ((0, ps0), (1, ps1)):
                        for ib in range(KB):
                            mm_bf16(ps, xb, ib, t)
                        for j in range(KF_PAIRS):
                            mm_fp8(ps, x8, j, t)
                evac_store(2 * pr, ps0)
                evac_store(2 * pr + 1, ps1)

    nc.compile()
    _NC_CACHE[tb_count] = nc
    return nc


def _shard_inputs(x, W):
    import ml_dtypes

    x = np.asarray(x)
    if x.dtype != np.float32:
        x = x.astype(np.float32)
    W = np.asarray(W)
    if W.dtype != ml_dtypes.bfloat16:
        W = W.astype(ml_dtypes.bfloat16)
    n_pairs = GROUP // (2 * P)
    in_maps = []
    for c in range(N_CORES):
        g, h = c // 2, c % 2
        xg = x[g * GROUP : (g + 1) * GROUP]
        xt = np.ascontiguousarray(xg.reshape(n_pairs, 2 * P, HIDDEN).transpose(0, 2, 1))
        in_maps.append(
            {
                "x": xt,
                "w": np.ascontiguousarray(W[g, h * O_HALF : (h + 1) * O_HALF, :].T),
            }
        )
    return in_maps


def kernel(x, W, group_sizes=None, **_ignored):
    if group_sizes is not None:
        gs = np.asarray(group_sizes).astype(np.int64)
        assert gs.shape == (NUM_EXPERTS,) and np.all(gs == GROUP)
    _ensure_paths()
    from concourse.bass_utils import run_bass_kernel_spmd

    nc = build_nc()
    in_maps = _shard_inputs(x, W)
    res = run_bass_kernel_spmd(nc, in_maps, core_ids=list(range(N_CORES)))
    y = np.empty((TOTAL, HIDDEN), dtype=np.float32)
    for c in range(N_CORES):
        g, h = c // 2, c % 2
        y[g * GROUP : (g + 1) * GROUP, h * O_HALF : (h + 1) * O_HALF] = res.results[c]["y"]
    return y


# revision 3
# speedup vs baseline: 1.0004x; 1.0004x over previous
"""Hybrid split-K bf16 + fp8(e4m3 DoubleRow) grouped GEMM for TRN2.

y[t] = x[t] @ W[g].T with the contraction split: first KB k-blocks (128 each)
in bf16 (1 cy/row), last KF_PAIRS k-pairs (256 each) in fp8 e4m3 DoubleRow
(2x FLOP rate, 157 TF/s). All products accumulate in the same PSUM banks at a
common power-of-2 scale 4096*y:
    bf16 side: lhsT = bf16(4096*x)  (exact pow2, equals 4096*bf16(x)),
               rhs  = raw bf16 W^T tiles (DMA'd straight to SBUF, no cast)
    fp8 side:  lhsT = e4m3(16*x), rhs = e4m3(256*W^T)    (16*256 = 4096)
One ACT copy with scale 2^-12 evacuates PSUM (exact).

Quantization error (deterministic for the fixed jax key-0 inputs) comes only
from the fp8 fraction: rel_err = 3.15e-2 * sqrt(KF_PAIRS/16).

Sharding (8 cores): expert-parallel x output-column-parallel, same as the
bf16 baseline; host-side transforms are layout-only.
"""

import os
import sys

import numpy as np

NUM_EXPERTS = 4
GROUP = 4096
HIDDEN = 4096
TOTAL = NUM_EXPERTS * GROUP
N_CORES = 8
O_HALF = HIDDEN // 2

P = 128
IB = HIDDEN // P          # 32 k-blocks
KF_PAIRS = 4              # fp8 k-pairs (256 k each) at the tail of K
KB = IB - 2 * KF_PAIRS    # bf16 k-blocks
NB = 512                  # bf16 matmul moving free dim (one PSUM bank)
OB = O_HALF // NB         # 4 psum banks per token block
U = 256                   # tokens per pair slab
SB = 4096.0               # bf16 x scale
SX = 16.0                 # fp8 x scale
SW = 256.0                # fp8 W scale


def _ensure_paths():
    for p in ("/opt/trn_rl_repo", "/root/.axon_site", "/root/.axon_site/_ro/pypackages"):
        if os.path.isdir(p) and p not in sys.path:
            sys.path.append(p)
    try:
        import concourse  # noqa: F401
    except ImportError:
        raise RuntimeError("concourse not importable; check PYTHONPATH")


_NC_CACHE = {}


def build_nc(tb_count=GROUP // P):
    if tb_count in _NC_CACHE:
        return _NC_CACHE[tb_count]
    _ensure_paths()
    import concourse.mybir as mybir
    import concourse.tile as tile
    from concourse import bacc

    DR = mybir.MatmulPerfMode.DoubleRow
    assert tb_count % 2 == 0
    n_pairs = tb_count // 2
    CHUNKS = (2, 6, 12, 12)
    assert sum(CHUNKS) == IB

    nc = bacc.Bacc("TRN2", target_bir_lowering=False, debug=False)
    x_d = nc.dram_tensor("x", [n_pairs, HIDDEN, U], mybir.dt.float32, kind="ExternalInput")
    w_d = nc.dram_tensor("w", [HIDDEN, O_HALF], mybir.dt.bfloat16, kind="ExternalInput")
    y_d = nc.dram_tensor("y", [tb_count * P, O_HALF], mybir.dt.float32, kind="ExternalOutput")
    x_ap, w_ap, y_ap = x_d.ap(), w_d.ap(), y_d.ap()

    with tile.TileContext(nc) as tc:
        from contextlib import ExitStack

        with ExitStack() as ctx:
            wt_pool = ctx.enter_context(tc.tile_pool(name="wt", bufs=1))
            w8s_pool = ctx.enter_context(tc.tile_pool(name="w8s", bufs=2))
            w8_pool = ctx.enter_context(tc.tile_pool(name="w8", bufs=1))
            xf_pool = ctx.enter_context(tc.tile_pool(name="xf", bufs=2))
            xb_pool = ctx.enter_context(tc.tile_pool(name="xb", bufs=2))
            x8_pool = ctx.enter_context(tc.tile_pool(name="x8", bufs=2))
            out_pool = ctx.enter_context(tc.tile_pool(name="out", bufs=2))
            psum_pool = ctx.enter_context(tc.tile_pool(name="psum", bufs=1, space="PSUM"))

            # ---- W bf16 resident: KB tiles [128, O_HALF], raw DMA, no cast.
            # Emission is interleaved with pair-0's later x chunks below so the
            # scalar DMA queue alone carries the whole HBM-bound prologue and W
            # arrival keeps pace with the PE's K-major consumption. ----
            wT = []
            for ib in range(KB):
                t = wt_pool.tile([P, O_HALF], mybir.dt.bfloat16, name=f"wT{ib}", tag=f"wT{ib}")
                wT.append(t)

            def dma_wT(lo, hi):
                for ib in range(lo, min(hi, KB)):
                    nc.scalar.dma_start(wT[ib][:], w_ap[ib * P : (ib + 1) * P, :])

            w8st = []

            def dma_w8st():
                # rides the sync queue, which is idle during the W prologue
                for j in range(KF_PAIRS):
                    st = w8s_pool.tile([P, 2, O_HALF], mybir.dt.bfloat16, name=f"w8s{j}", tag="w8s")
                    r0 = (KB + 2 * j) * P
                    nc.sync.dma_start(st[:], w_ap[r0 : r0 + 2 * P, :].rearrange("(j p) o -> p j o", p=P))
                    w8st.append(st)

            w8 = []

            def cast_w8():
                for j in range(KF_PAIRS):
                    t = w8_pool.tile([P, 2, O_HALF], mybir.dt.float8e4, name=f"w8_{j}", tag=f"w8_{j}")
                    nc.vector.tensor_scalar_mul(t[:], w8st[j][:], SW)
                    w8.append(t)

            def load_cast(pr):
                """DMA one pair slab (ib-chunked), split-cast: bf16(4096x) head,
                e4m3(16x) tail. Pair 0 interleaves its later chunks into the
                scalar queue between W tiles (prologue HBM scheduling); pair 1
                rides the scalar queue FIFO-behind the remaining W stream."""
                src = x_ap[pr].rearrange("(ib p) u -> p ib u", p=P)
                xb = xb_pool.tile([P, KB, U], mybir.dt.bfloat16, name=f"xb_{pr}", tag="xb")
                x8 = x8_pool.tile([P, 2 * KF_PAIRS, U], mybir.dt.float8e4, name=f"x8_{pr}", tag="x8")
                s0 = 0
                for c, ch in enumerate(CHUNKS):
                    if pr == 0:
                        if c == 0:
                            dma_wT(0, 2)
                            eng = nc.sync
                        elif c == 1:
                            eng = nc.sync
                        elif c == 2:
                            dma_wT(2, 8)
                            eng = nc.scalar
                        else:
                            dma_wT(8, 16)
                            eng = nc.scalar
                    else:
                        eng = nc.scalar if pr == 1 else nc.sync
                    xf = xf_pool.tile([P, max(CHUNKS), U], mybir.dt.float32, name=f"xf_{pr}_{c}", tag="xf")
                    eng.dma_start(xf[:, :ch, :], src[:, s0 : s0 + ch, :])
                    if s0 < KB:
                        b_end = min(KB, s0 + ch)
                        nc.vector.tensor_scalar_mul(xb[:, s0:b_end, :], xf[:, : b_end - s0, :], SB)
                    if s0 + ch > KB:
                        f0 = max(KB, s0)
                        nc.vector.tensor_scalar_mul(
                            x8[:, f0 - KB : s0 + ch - KB, :], xf[:, f0 - s0 : ch, :], SX
                        )
                    if pr == 0 and c == 1:
                        dma_w8st()
                    s0 += ch
                if pr == 0:
                    dma_wT(16, KB)
                return xb, x8

            def alloc_psum(tb):
                grp = (tb % 2) * OB
                return [
                    psum_pool.tile([P, NB], mybir.dt.float32, name=f"ps_{tb}_{ob}", tag=f"bank{grp + ob}")
                    for ob in range(OB)
                ]

            def mm_bf16(ps, xb, ib, t):
                lhsT = xb[:, ib, t * P : (t + 1) * P]
                for ob in range(OB):
                    nc.tensor.matmul(
                        ps[ob][:], lhsT, wT[ib][:, ob * NB : (ob + 1) * NB],
                        start=(ib == 0), stop=False,
                    )

            def mm_fp8(ps, x8, j, t):
                lhsT = x8[:, 2 * j : 2 * j + 2, t * P : (t + 1) * P]
                last = j == KF_PAIRS - 1
                for ob in range(OB):
                    for h in range(2):
                        nc.tensor.matmul(
                            ps[ob][:, h * 256 : (h + 1) * 256],
                            lhsT,
                            w8[j][:, :, (2 * ob + h) * 256 : (2 * ob + h + 1) * 256],
                            start=False,
                            stop=(last and h == 1),
                            perf_mode=DR,
                        )

            def evac_store(tb, ps):
                yo = out_pool.tile([P, O_HALF], mybir.dt.float32, name=f"yo_{tb}", tag="yo")
                for ob in range(OB):
                    # per-bank copy + DMA so the store tail overlaps compute
                    nc.scalar.mul(yo[:, ob * NB : (ob + 1) * NB], ps[ob][:], 1.0 / SB)
                    nc.sync.dma_start(
                        y_ap[tb * P : (tb + 1) * P, ob * NB : (ob + 1) * NB],
                        yo[:, ob * NB : (ob + 1) * NB],
                    )

            for pr in range(n_pairs):
                xb, x8 = load_cast(pr)
                if pr == 0:
                    cast_w8()  # DVE: after pair-0 x casts, before pair-1
                ps0 = alloc_psum(2 * pr)
                ps1 = alloc_psum(2 * pr + 1)
                if pr == 0:
                    # K-major interleave so PE weight-tile consumption matches
                    # the W stream arrival rate during the HBM-bound prologue.
                    for ib in range(KB):
                        mm_bf16(ps0, xb, ib, 0)
                        mm_bf16(ps1, xb, ib, 1)
                    for j in range(KF_PAIRS):
                        mm_fp8(ps0, x8, j, 0)
                        mm_fp8(ps1, x8, j, 1)
                else:
                    for t, ps in ((0, ps0), (1, ps1)):
                        for ib in range(KB):
                            mm_bf16(ps, xb, ib, t)
                        for j in range(KF_PAIRS):
                            mm_fp8(ps, x8, j, t)
                evac_store(2 * pr, ps0)
                evac_store(2 * pr + 1, ps1)

    nc.compile()
    _NC_CACHE[tb_count] = nc
    return nc


def _shard_inputs(x, W):
    import ml_dtypes

    x = np.asarray(x)
    if x.dtype != np.float32:
        x = x.astype(np.float32)
    W = np.asarray(W)
    if W.dtype != ml_dtypes.bfloat16:
        W = W.astype(ml_dtypes.bfloat16)
    n_pairs = GROUP // (2 * P)
    in_maps = []
    for c in range(N_CORES):
        g, h = c // 2, c % 2
        xg = x[g * GROUP : (g + 1) * GROUP]
        xt = np.ascontiguousarray(xg.reshape(n_pairs, 2 * P, HIDDEN).transpose(0, 2, 1))
        in_maps.append(
            {
                "x": xt,
                "w": np.ascontiguousarray(W[g, h * O_HALF : (h + 1) * O_HALF, :].T),
            }
        )
    return in_maps


def kernel(x, W, group_sizes=None, **_ignored):
    if group_sizes is not None:
        gs = np.asarray(group_sizes).astype(np.int64)
        assert gs.shape == (NUM_EXPERTS,) and np.all(gs == GROUP)
    _ensure_paths()
    from concourse.bass_utils import run_bass_kernel_spmd

    nc = build_nc()
    in_maps = _shard_inputs(x, W)
    res = run_bass_kernel_spmd(nc, in_maps, core_ids=list(range(N_CORES)))
    y = np.empty((TOTAL, HIDDEN), dtype=np.float32)
    for c in range(N_CORES):
        g, h = c // 2, c % 2
        y[g * GROUP : (g + 1) * GROUP, h * O_HALF : (h + 1) * O_HALF] = res.results[c]["y"]
    return y
